# revision 2
# baseline (speedup 1.0000x reference)
"""Trainium2 Bass kernel for nn_MemoryPlus (retrieval_knn).

Strategy (8 NeuronCores, data-parallel over the 4096 tokens, 512/core):
  sims = q @ k_norm^T runs in plain fp32 on the PE: top-32-of-32768
  selection gaps are ~4e-4 absolute vs bf16 noise ~1.5e-4, so 16-bit
  (and fp32r) matmuls mis-select members and blow the 2e-2 error
  budget; a single boundary swap costs ~4e-3 global rel err.  The
  top-8 per 1024-shard scan (max8 + find_index8) runs on fp32 values
  straight out of PSUM.  The 256 candidates reduce to an exact top-32
  (max8/match_replace), softmax runs on rq-scaled logits, value rows
  are fetched with gpsimd dma_gather (bf16) on two software-DGE
  queues, and accumulated on the PE via diag(w_j) matmuls into PSUM
  (the per-token softmax weight is folded into a diagonal stationary
  matrix built on the Scalar engine - no full-width scaled copies).
  gate and the output projection run in bf16; y^T for the output
  matmul comes from a DMA XBAR transpose.

  Scheduling: tiles sweep the 32 key blocks in three groups (0,1) /
  (2,) / (3,) (keys streamed three times).  A finished group's tail
  (top-32 + softmax + gather-index staging + first gathers) issues at
  high priority, but its PE work (diag matmuls + output matmul) is
  deferred and drip-fed into the next sweep's block loop, one gather
  chunk per block, so the in-order PE stream never stalls on gather
  latency.  Only tile 3's tail is exposed at the end.

Host-side work is layout only (transposes / normalization / dtype
packing of fixed weights+inputs).
"""

import os

import ml_dtypes
import numpy as np

import concourse.bass as bass
import concourse.tile as tile
from concourse import bacc, mybir
from concourse.bass_utils import run_bass_kernel_spmd
from concourse.masks import make_identity

F32 = mybir.dt.float32
BF16 = mybir.dt.bfloat16
I16 = mybir.dt.int16
U16 = mybir.dt.uint16
AF = mybir.ActivationFunctionType
ALU = mybir.AluOpType

N_CORES = 8
NEG = -1.0e30


class Cfg:
    def __init__(self, n_mem=32768, n_ttiles=4, d_model=1024, d_key=256,
                 d_val=1024, k=32, block=1024, gjc=4):
        self.n_mem = n_mem
        self.n_ttiles = n_ttiles          # token tiles of 128 per core
        self.T = 128 * n_ttiles           # tokens per core
        self.d_model = d_model
        self.d_key = d_key
        self.d_val = d_val
        self.k = k
        self.block = block                # mem block per k DMA (= shard)
        self.n_blocks = n_mem // block
        self.n_cand = 8 * self.n_blocks   # top-8 per shard
        self.gjc = gjc                    # value-gather j-chunk
        assert self.n_cand >= k and k % 8 == 0


FULL = Cfg()


def build(cfg: Cfg):
    nc = bacc.Bacc("TRN2", target_bir_lowering=False, debug=False,
                   num_devices=N_CORES, num_swdge_queues=2)
    dm, dk, dv, T = cfg.d_model, cfg.d_key, cfg.d_val, cfg.T
    n_dm, n_dk, n_dv = dm // 128, dk // 128, dv // 128

    xT = nc.dram_tensor("xT", [128, n_dm, T], F32, kind="ExternalInput").ap()
    wqT = nc.dram_tensor("wqT", [128, n_dm, dk], F32,
                         kind="ExternalInput").ap()
    kpk = nc.dram_tensor("kpk", [cfg.n_blocks, 128, n_dk, cfg.block],
                         F32, kind="ExternalInput").ap()
    wg = nc.dram_tensor("wg", [128, n_dm, dv], BF16,
                        kind="ExternalInput").ap()
    wo = nc.dram_tensor("wo", [128, n_dv, dm], BF16,
                        kind="ExternalInput").ap()
    vals = nc.dram_tensor("vals", [cfg.n_mem, dv], BF16,
                          kind="ExternalInput").ap()
    shof = nc.dram_tensor("shof", [cfg.n_cand], F32,
                          kind="ExternalInput").ap()
    out = nc.dram_tensor("out", [T, dm], F32, kind="ExternalOutput").ap()
    stage = nc.dram_tensor("stage", [cfg.n_ttiles * cfg.k * 128], I16)
    nrmd = nc.dram_tensor("nrmd", [T], F32)

    with tile.TileContext(nc) as tc:
        _kernel_body(tc, cfg, xT, wqT, kpk, wg, wo, vals, shof, out,
                     stage, nrmd)
    nc.compile()
    return nc


def _kernel_body(tc, cfg, xT, wqT, kpk, wg, wo, vals, shof, out,
                 stage, nrmd):
    nc = tc.nc
    dm, dk, dv, T, K = cfg.d_model, cfg.d_key, cfg.d_val, cfg.T, cfg.k
    n_dm, n_dk, n_dv = dm // 128, dk // 128, dv // 128
    NT = cfg.n_ttiles
    NCD = cfg.n_cand
    NB = cfg.n_blocks

    with tc.tile_pool(name="persist", bufs=1) as persist:
        ident = persist.tile([128, 128], F32)
        make_identity(nc, ident)
        identb = persist.tile([128, 128], BF16)
        nc.vector.tensor_copy(identb, ident)

        # chunked input DMAs so qproj starts on the first 256KB
        xT_sb = persist.tile([128, n_dm, T], F32)
        for d in range(n_dm):
            nc.sync.dma_start(out=xT_sb[:, d, :], in_=xT[:, d, :])
        xb_sb = persist.tile([128, n_dm, T], BF16)
        nc.scalar.activation(xb_sb, xT_sb, AF.Copy)
        wg_sb = persist.tile([128, n_dm, dv], BF16)
        nc.sync.dma_start(out=wg_sb, in_=wg)
        wo_sb = persist.tile([128, n_dv, dm], BF16)
        nc.scalar.dma_start(out=wo_sb, in_=wo)
        shof_sb = persist.tile([128, NCD], F32)
        nc.scalar.dma_start(
            out=shof_sb,
            in_=bass.AP(tensor=shof.tensor, offset=0, ap=[[0, 128], [1, NCD]]))

        qT_sb = persist.tile([128, n_dk, T], F32)
        rq = persist.tile([128, NT], F32)
        candV = persist.tile([128, NT, NCD], F32)
        candP = persist.tile([128, NT, NCD], U16)
        gate_sb = persist.tile([128, NT, dv], BF16)

        # ---- phase A: qT = wq^T x (fp32) and rq = 1/|q| ----
        with tc.tile_pool(name="qphase", bufs=1) as qp, \
             tc.tile_pool(name="qps", bufs=2, space="PSUM") as qps:
            wq_sb = qp.tile([128, n_dm, dk], F32, tag="wq")
            for d in range(n_dm):
                nc.sync.dma_start(out=wq_sb[:, d, :], in_=wqT[:, d, :])
            for ckp in range(n_dk):
                ps = qps.tile([128, T], F32, tag="qmm")
                for d in range(n_dm):
                    nc.tensor.matmul(ps,
                                     wq_sb[:, d, 128 * ckp:128 * (ckp + 1)],
                                     xT_sb[:, d, :],
                                     start=(d == 0), stop=(d == n_dm - 1))
                nc.scalar.activation(qT_sb[:, ckp, :], ps, AF.Copy)

            # |q|^2 per token via ones-matmul; DRAM round-trip to [128, NT]
            sq = qp.tile([128, n_dk, T], F32, tag="sq")
            nc.scalar.activation(sq, qT_sb, AF.Square)
            ones = qp.tile([128, 1], F32, tag="ones")
            nc.vector.memset(ones, 1.0)
            psn = qps.tile([1, T], F32, tag="qnrm")
            for ckp in range(n_dk):
                nc.tensor.matmul(psn, ones, sq[:, ckp, :],
                                 start=(ckp == 0), stop=(ckp == n_dk - 1))
            nrm_sb = qp.tile([1, T], F32, tag="nrm")
            nc.scalar.activation(nrm_sb, psn, AF.Copy)
            nc.sync.dma_start(
                out=bass.AP(tensor=nrmd, offset=0, ap=[[1, T]]), in_=nrm_sb)
            nrm2 = qp.tile([128, NT], F32, tag="nrm2")
            nc.sync.dma_start(
                out=nrm2,
                in_=bass.AP(tensor=nrmd, offset=0, ap=[[1, 128], [128, NT]]))
            nrms = qp.tile([128, NT], F32, tag="nrms")
            nc.scalar.activation(nrms, nrm2, AF.Sqrt)
            nc.vector.reciprocal(rq, nrms)

        # ---- phase B: sims + scan per group sweep; tails drip into the
        # next sweep as per-chunk closures so the PE never stalls ----
        with tc.tile_pool(name="kbp", bufs=3) as kbp, \
             tc.tile_pool(name="evp", bufs=2) as evp, \
             tc.tile_pool(name="tailp", bufs=1) as tp, \
             tc.tile_pool(name="gathp", bufs=4) as gp, \
             tc.tile_pool(name="diagp", bufs=4) as dgp, \
             tc.tile_pool(name="gop", bufs=2) as gop, \
             tc.tile_pool(name="simps", bufs=2, space="PSUM") as sps, \
             tc.tile_pool(name="dps", bufs=2, space="PSUM") as dps:
            groups = [(0, 1), (2,), (3,)]
            pending = []          # deferred tail-PE closures
            for gi, tiles in enumerate(groups):
                for b in range(NB):
                    kb = kbp.tile([128, n_dk, cfg.block], F32, tag="kb",
                                  name="kb")
                    nc.sync.dma_start(out=kb, in_=kpk[b, :, :, :])
                    for i in tiles:
                        nch = cfg.block // 512
                        pss = sps.tile([128, cfg.block], F32, tag="sim",
                                       name="sim")
                        for ckp in range(n_dk):
                            qch = qT_sb[:, ckp, 128 * i:128 * (i + 1)]
                            for c2 in range(nch):
                                sl = slice(512 * c2, 512 * (c2 + 1))
                                nc.tensor.matmul(pss[:, sl], qch,
                                                 kb[:, ckp, sl],
                                                 start=(ckp == 0),
                                                 stop=(ckp == n_dk - 1),
                                                 skip_group_check=True)
                        nc.vector.max(candV[:, i, 8 * b:8 * b + 8], pss)
                        nc.vector.max_index(candP[:, i, 8 * b:8 * b + 8],
                                            candV[:, i, 8 * b:8 * b + 8],
                                            pss)
                    if b >= 1 and pending:
                        pending.pop(0)()
                    if b == NB // 2:
                        for i in tiles:
                            _gate_tile(tc, cfg, i, xb_sb, wg_sb, gate_sb,
                                       sps)
                # ---- end of sweep: tails for this group's tiles ----
                for i in tiles:
                    with tc.high_priority():
                        w32 = _tail(tc, cfg, i, candV, candP, shof_sb,
                                    rq, stage, tp)
                        st = _gather_start(tc, cfg, i, w32, vals, stage,
                                           tp, gp)
                    pending.extend(
                        _out_closures(tc, cfg, i, st, vals, identb,
                                      gate_sb, wo_sb, out, tp, gp, dgp,
                                      gop, dps))
            for fn in pending:
                fn()


def _gate_tile(tc, cfg, i, xb_sb, wg_sb, gate_sb, sps):
    nc = tc.nc
    n_dm = cfg.d_model // 128
    psg = sps.tile([128, cfg.d_val], F32, tag="sim", name="psg")
    for d in range(n_dm):
        xch = xb_sb[:, d, 128 * i:128 * (i + 1)]
        for h in range(2):
            sl = slice(512 * h, 512 * (h + 1))
            nc.tensor.matmul(psg[:, sl], xch, wg_sb[:, d, sl],
                             start=(d == 0), stop=(d == n_dm - 1),
                             skip_group_check=True)
    # silu(x) = x * sigmoid(x) exactly, matching the reference
    nc.scalar.activation(gate_sb[:, i, :], psg, AF.Sigmoid)
    nc.vector.tensor_mul(gate_sb[:, i, :], gate_sb[:, i, :], psg)


def _tail(tc, cfg, i, candV, candP, shof_sb, rq, stage, tp):
    """Exact top-32 + softmax weights + gather-index staging for tile i."""
    nc = tc.nc
    K, NCD = cfg.k, cfg.n_cand

    scr = tp.tile([128, NCD], F32, tag="scr", name="scr")
    nc.vector.tensor_copy(scr, candV[:, i, :])
    mx = tp.tile([128, K], F32, tag="mx", name="mx")
    for r in range(K // 8):
        nc.vector.max(mx[:, 8 * r:8 * r + 8], scr)
        if r < K // 8 - 1:
            nc.vector.match_replace(scr, mx[:, 8 * r:8 * r + 8], scr, NEG)
    t1 = mx[:, K - 1:K]

    mask = tp.tile([128, NCD], F32, tag="mask", name="mask")
    nc.vector.tensor_scalar(mask, candV[:, i, :], t1, None, ALU.is_ge)
    pfull = tp.tile([128, NCD], F32, tag="pfull", name="pfull")
    nc.vector.tensor_copy(pfull, candP[:, i, :])
    nc.vector.tensor_add(pfull, pfull, shof_sb)
    pfm = tp.tile([128, NCD], F32, tag="pfm", name="pfm")
    nc.vector.tensor_mul(pfm, pfull, mask)

    g32 = tp.tile([128, K], F32, tag="g32", name="g32")
    for r in range(K // 8):
        nc.vector.max(g32[:, 8 * r:8 * r + 8], pfm)
        if r < K // 8 - 1:
            nc.vector.match_replace(pfm, g32[:, 8 * r:8 * r + 8], pfm, 0.0)
    idx16 = tp.tile([128, K], I16, tag="idx16", name="idx16")
    nc.vector.tensor_scalar(idx16, g32, 1.0, None, ALU.subtract)

    # stage j-major to DRAM immediately -- the gather chain (stage -> wr ->
    # dma_gather) is latency-bound and must start before eqscr/softmax
    # write pre-wrapped: stage[(p%16)*256 + (p//16) + 8*j] = idx16[p, j],
    # so the gather-index readback is contiguous 512B per partition
    nc.sync.dma_start(
        out=bass.AP(tensor=stage, offset=i * K * 128,
                    ap=[[1, 8], [8 * K, 16], [8, K]]),
        in_=idx16)

    # v32[j] = candV at the slot whose (pos+shard offset) == g32[j]
    eqscr = tp.tile([128, NCD], F32, tag="eqscr", name="eqscr")
    v32 = tp.tile([128, K], F32, tag="v32", name="v32")
    for j in range(K):
        nc.vector.scalar_tensor_tensor(eqscr, pfull, g32[:, j:j + 1],
                                       candV[:, i, :], op0=ALU.is_equal,
                                       op1=ALU.mult,
                                       accum_out=v32[:, j:j + 1])

    # softmax over rq * v32; mx[:,0] is the max logit pre-scale
    bexp = tp.tile([128, 1], F32, tag="bexp", name="bexp")
    nc.vector.scalar_tensor_tensor(bexp, mx[:, 0:1], -1.0, rq[:, i:i + 1],
                                   op0=ALU.mult, op1=ALU.mult)
    e32 = tp.tile([128, K], F32, tag="e32", name="e32")
    ssum = tp.tile([128, 1], F32, tag="ssum", name="ssum")
    nc.scalar.activation(e32, v32, AF.Exp, bias=bexp, scale=rq[:, i:i + 1],
                         accum_out=ssum)
    rs = tp.tile([128, 1], F32, tag="rs", name="rs")
    nc.vector.reciprocal(rs, ssum)
    w32 = tp.tile([128, K], F32, tag=f"w32_{i}", name=f"w32_{i}")
    nc.vector.tensor_scalar(w32, e32, rs, None, ALU.mult)
    return w32


def _gather_start(tc, cfg, i, w32, vals, stage, tp, gp):
    """Index readback + first two value-gather chunks for tile i."""
    nc = tc.nc
    K, dv = cfg.k, cfg.d_val

    wr = tp.tile([128, 8 * K], I16, tag=f"wr_{i}", name=f"wr_{i}")
    nc.sync.dma_start(
        out=wr,
        in_=bass.AP(tensor=stage, offset=i * K * 128,
                    ap=[[0, 8], [8 * K, 16], [1, 8 * K]]))

    vgs = {}
    for jc in range(2):
        vg = gp.tile([128, cfg.gjc, dv], BF16, tag="vg", name="vg")
        nc.gpsimd.dma_gather(
            vg, vals, wr[:, 8 * cfg.gjc * jc:8 * cfg.gjc * (jc + 1)],
            num_idxs=128 * cfg.gjc, num_idxs_reg=128 * cfg.gjc,
            elem_size=dv, queue_num=jc % 2)
        vgs[jc] = vg
    return {"wr": wr, "w32": w32, "vgs": vgs}


def _out_closures(tc, cfg, i, st, vals, identb, gate_sb, wo_sb, out, tp,
                  gp, dgp, gop, dps):
    """Deferred PE work for tile i: per-chunk diag(w) matmuls into PSUM,
    then gate multiply + transpose + output projection."""
    nc = tc.nc
    dm, dv, K = cfg.d_model, cfg.d_val, cfg.k
    n_dv = dv // 128
    n_chunks = K // cfg.gjc
    wr, w32, vgs = st["wr"], st["w32"], st["vgs"]
    psm_box = {}

    def chunk_closure(c):
        def run():
            if c == 0:
                psm_box["psm"] = dps.tile([128, dv], F32, tag="m512",
                                          name="psm")
            nxt = c + 2
            if nxt < n_chunks:
                vg = gp.tile([128, cfg.gjc, dv], BF16, tag="vg", name="vg")
                nc.gpsimd.dma_gather(
                    vg, vals,
                    wr[:, 8 * cfg.gjc * nxt:8 * cfg.gjc * (nxt + 1)],
                    num_idxs=128 * cfg.gjc, num_idxs_reg=128 * cfg.gjc,
                    elem_size=dv, queue_num=nxt % 2)
                vgs[nxt] = vg
            psm = psm_box["psm"]
            vg = vgs.pop(c)
            for jj in range(cfg.gjc):
                j = cfg.gjc * c + jj
                diag = dgp.tile([128, 128], BF16, tag="diag", name="diag")
                nc.scalar.activation(diag, identb, AF.Copy,
                                     scale=w32[:, j:j + 1])
                for h in range(2):
                    sl = slice(512 * h, 512 * (h + 1))
                    nc.tensor.matmul(psm[:, sl], diag, vg[:, jj, sl],
                                     start=(j == 0), stop=(j == K - 1),
                                     skip_group_check=True)
        return run

    def finish():
        psm = psm_box["psm"]
        # y = mem * gate (bf16), reading mem straight out of PSUM
        y = gop.tile([128, dv], BF16, tag="y", name="y")
        nc.vector.tensor_mul(y, psm, gate_sb[:, i, :])

        yT = gop.tile([128, n_dv, 128], BF16, tag="yT", name="yT")
        nc.sync.dma_start(out=yT, in_=y, transpose=True)
        out_sb = gop.tile([128, dm], F32, tag="outsb", name="outsb")
        pso = dps.tile([128, dm], F32, tag="m512", name="pso")
        for v in range(n_dv):
            for h in range(2):
                sl = slice(512 * h, 512 * (h + 1))
                nc.tensor.matmul(pso[:, sl], yT[:, v, :],
                                 wo_sb[:, v, sl],
                                 start=(v == 0), stop=(v == n_dv - 1),
                                 skip_group_check=True)
        nc.scalar.activation(out_sb, pso, AF.Copy)
        nc.sync.dma_start(out=out[128 * i:128 * (i + 1), :], in_=out_sb)

    return [chunk_closure(c) for c in range(n_chunks)] + [finish]


# ---------------------------------------------------------------- host side

_CACHE = {}


def _prep(x, keys, values, w_q, w_gate, w_out, cfg):
    dm, dk, dv = cfg.d_model, cfg.d_key, cfg.d_val
    n_dm, n_dk, n_dv = dm // 128, dk // 128, dv // 128
    xf = np.ascontiguousarray(x.reshape(-1, dm)).astype(np.float32)

    norm = np.sqrt((keys.astype(np.float64) ** 2).sum(1, keepdims=True))
    knm = (keys / np.maximum(norm, 1e-12)).astype(np.float32)
    knT = np.ascontiguousarray(knm.T)             # [dk, n_mem]
    r = knT.reshape(n_dk, 128, cfg.n_blocks, cfg.block)
    kpk = np.ascontiguousarray(r.transpose(2, 1, 0, 3))

    wqT = np.ascontiguousarray(w_q.T)             # [dm, dk]
    wqp = np.ascontiguousarray(
        wqT.reshape(n_dm, 128, dk).transpose(1, 0, 2))

    wgT = np.ascontiguousarray(w_gate.T)          # [dm, dv]
    wgp = wgT.astype(ml_dtypes.bfloat16).reshape(n_dm, 128, dv)
    wgp = np.ascontiguousarray(wgp.transpose(1, 0, 2))
    woT = np.ascontiguousarray(w_out.T)           # [dv, dm]
    wop = woT.astype(ml_dtypes.bfloat16).reshape(n_dv, 128, dm)
    wop = np.ascontiguousarray(wop.transpose(1, 0, 2))

    shof = ((np.arange(cfg.n_cand, dtype=np.float32) // 8) * cfg.block
            + 1.0).astype(np.float32)
    common = {
        "kpk": kpk,
        "vals": np.ascontiguousarray(values).astype(ml_dtypes.bfloat16),
        "wqT": wqp,
        "wg": wgp,
        "wo": wop,
        "shof": shof,
    }
    in_maps = []
    for c in range(N_CORES):
        xc = xf[c * cfg.T:(c + 1) * cfg.T]        # [T, dm]
        xTc = np.ascontiguousarray(xc.T)          # [dm, T]
        xp = np.ascontiguousarray(
            xTc.reshape(n_dm, 128, cfg.T).transpose(1, 0, 2))
        m = dict(common)
        m["xT"] = xp
        in_maps.append(m)
    return in_maps


def kernel(x, keys, values, w_q, w_gate, w_out):
    cfg = FULL
    if "nc" not in _CACHE:
        _CACHE["nc"] = build(cfg)
    nc = _CACHE["nc"]
    x = np.asarray(x)
    in_maps = _prep(x, np.asarray(keys), np.asarray(values),
                    np.asarray(w_q), np.asarray(w_gate), np.asarray(w_out),
                    cfg)
    trace = os.environ.get("KERNEL_TRACE", "0") == "1"
    if trace:
        try:
            import ntff_shim
            ntff_shim.install()
        except Exception:
            pass
    res = run_bass_kernel_spmd(nc, in_maps, list(range(N_CORES)), trace=trace)
    if trace:
        _CACHE["exec_time_ns"] = res.exec_time_ns
    outs = [res.results[c]["out"] for c in range(N_CORES)]
    B, S, D = x.shape
    return np.concatenate(outs, axis=0).reshape(B, S, D)


# revision 10
# speedup vs baseline: 1.1180x; 1.1180x over previous
"""Trainium2 Bass kernel for nn_MemoryPlus (retrieval_knn).

Strategy (8 NeuronCores, data-parallel over the 4096 tokens, 512/core):
  sims = q @ k_norm^T runs in plain fp32 on the PE: top-32-of-32768
  selection gaps are ~4e-4 absolute vs bf16 noise ~1.5e-4, so 16-bit
  (and fp32r) matmuls mis-select members and blow the 2e-2 error
  budget; a single boundary swap costs ~4e-3 global rel err.  The
  top-8 per 1024-shard scan (max8 + find_index8) runs on fp32 values
  straight out of PSUM.  The 256 candidates reduce to an exact top-32
  (max8/match_replace), softmax runs on rq-scaled logits, value rows
  are fetched with gpsimd dma_gather (bf16) on two software-DGE
  queues, and accumulated on the PE via diag(w_j) matmuls into PSUM
  (the per-token softmax weight is folded into a diagonal stationary
  matrix built on the Scalar engine - no full-width scaled copies).
  gate and the output projection run in bf16; y^T for the output
  matmul comes from a DMA XBAR transpose.

  Scheduling: tiles sweep the 32 key blocks in three groups (0,1) /
  (2,) / (3,) (keys streamed three times).  A finished group's tail
  (top-32 + softmax + gather-index staging + first gathers) issues at
  high priority, but its PE work (diag matmuls + output matmul) is
  deferred and drip-fed into the next sweep's block loop, one gather
  chunk per block, so the in-order PE stream never stalls on gather
  latency.  Only tile 3's tail is exposed at the end.

Host-side work is layout only (transposes / normalization / dtype
packing of fixed weights+inputs).
"""

import os

import ml_dtypes
import numpy as np

import concourse.bass as bass
import concourse.tile as tile
from concourse import bacc, mybir
from concourse.bass_utils import run_bass_kernel_spmd
from concourse.masks import make_identity

F32 = mybir.dt.float32
BF16 = mybir.dt.bfloat16
I16 = mybir.dt.int16
U16 = mybir.dt.uint16
AF = mybir.ActivationFunctionType
ALU = mybir.AluOpType

N_CORES = 8
NEG = -1.0e30


class Cfg:
    def __init__(self, n_mem=32768, n_ttiles=4, d_model=1024, d_key=256,
                 d_val=1024, k=32, block=1024, gjc=4):
        self.n_mem = n_mem
        self.n_ttiles = n_ttiles          # token tiles of 128 per core
        self.T = 128 * n_ttiles           # tokens per core
        self.d_model = d_model
        self.d_key = d_key
        self.d_val = d_val
        self.k = k
        self.block = block                # mem block per k DMA (= shard)
        self.n_blocks = n_mem // block
        self.n_cand = 8 * self.n_blocks   # top-8 per shard
        self.gjc = gjc                    # value-gather j-chunk
        assert self.n_cand >= k and k % 8 == 0


FULL = Cfg()


def build(cfg: Cfg):
    nc = bacc.Bacc("TRN2", target_bir_lowering=False, debug=False,
                   num_devices=N_CORES, num_swdge_queues=2)
    dm, dk, dv, T = cfg.d_model, cfg.d_key, cfg.d_val, cfg.T
    n_dm, n_dk, n_dv = dm // 128, dk // 128, dv // 128

    xT = nc.dram_tensor("xT", [128, n_dm, T], F32, kind="ExternalInput").ap()
    wqT = nc.dram_tensor("wqT", [128, n_dm, dk], F32,
                         kind="ExternalInput").ap()
    kpk = nc.dram_tensor("kpk", [cfg.n_blocks, 128, n_dk, cfg.block],
                         F32, kind="ExternalInput").ap()
    wg = nc.dram_tensor("wg", [128, n_dm, dv], BF16,
                        kind="ExternalInput").ap()
    wo = nc.dram_tensor("wo", [128, n_dv, dm], BF16,
                        kind="ExternalInput").ap()
    vals = nc.dram_tensor("vals", [cfg.n_mem, dv], BF16,
                          kind="ExternalInput").ap()
    shof = nc.dram_tensor("shof", [cfg.n_cand], F32,
                          kind="ExternalInput").ap()
    out = nc.dram_tensor("out", [T, dm], F32, kind="ExternalOutput").ap()
    stage = nc.dram_tensor("stage", [cfg.n_ttiles * cfg.k * 128], I16)
    nrmd = nc.dram_tensor("nrmd", [T], F32)

    with tile.TileContext(nc) as tc:
        _kernel_body(tc, cfg, xT, wqT, kpk, wg, wo, vals, shof, out,
                     stage, nrmd)
    nc.compile()
    return nc


def _kernel_body(tc, cfg, xT, wqT, kpk, wg, wo, vals, shof, out,
                 stage, nrmd):
    nc = tc.nc
    dm, dk, dv, T, K = cfg.d_model, cfg.d_key, cfg.d_val, cfg.T, cfg.k
    n_dm, n_dk, n_dv = dm // 128, dk // 128, dv // 128
    NT = cfg.n_ttiles
    NCD = cfg.n_cand
    NB = cfg.n_blocks

    with tc.tile_pool(name="persist", bufs=1) as persist:
        ident = persist.tile([128, 128], F32)
        make_identity(nc, ident)
        identb = persist.tile([128, 128], BF16)
        nc.vector.tensor_copy(identb, ident)

        # chunked input DMAs so qproj starts on the first 256KB
        xT_sb = persist.tile([128, n_dm, T], F32)
        for d in range(n_dm):
            nc.sync.dma_start(out=xT_sb[:, d, :], in_=xT[:, d, :])
        xb_sb = persist.tile([128, n_dm, T], BF16)
        nc.scalar.activation(xb_sb, xT_sb, AF.Copy)
        wg_sb = persist.tile([128, n_dm, dv], BF16)
        nc.sync.dma_start(out=wg_sb, in_=wg)
        wo_sb = persist.tile([128, n_dv, dm], BF16)
        nc.scalar.dma_start(out=wo_sb, in_=wo)
        shof_sb = persist.tile([128, NCD], F32)
        nc.scalar.dma_start(
            out=shof_sb,
            in_=bass.AP(tensor=shof.tensor, offset=0, ap=[[0, 128], [1, NCD]]))

        qT_sb = persist.tile([128, n_dk, T], F32)
        rq = persist.tile([128, NT], F32)
        candV = persist.tile([128, NT, NCD], F32)
        candP = persist.tile([128, NT, NCD], U16)
        gate_sb = persist.tile([128, NT, dv], BF16)

        # ---- phase A: qT = wq^T x (fp32) and rq = 1/|q| ----
        with tc.tile_pool(name="qphase", bufs=1) as qp, \
             tc.tile_pool(name="qps", bufs=2, space="PSUM") as qps:
            wq_sb = qp.tile([128, n_dm, dk], F32, tag="wq")
            for d in range(n_dm):
                nc.sync.dma_start(out=wq_sb[:, d, :], in_=wqT[:, d, :])
            for ckp in range(n_dk):
                ps = qps.tile([128, T], F32, tag="qmm")
                for d in range(n_dm):
                    nc.tensor.matmul(ps,
                                     wq_sb[:, d, 128 * ckp:128 * (ckp + 1)],
                                     xT_sb[:, d, :],
                                     start=(d == 0), stop=(d == n_dm - 1))
                nc.scalar.activation(qT_sb[:, ckp, :], ps, AF.Copy)

            # |q|^2 per token via ones-matmul; DRAM round-trip to [128, NT]
            sq = qp.tile([128, n_dk, T], F32, tag="sq")
            nc.scalar.activation(sq, qT_sb, AF.Square)
            ones = qp.tile([128, 1], F32, tag="ones")
            nc.vector.memset(ones, 1.0)
            psn = qps.tile([1, T], F32, tag="qnrm")
            for ckp in range(n_dk):
                nc.tensor.matmul(psn, ones, sq[:, ckp, :],
                                 start=(ckp == 0), stop=(ckp == n_dk - 1))
            nrm_sb = qp.tile([1, T], F32, tag="nrm")
            nc.scalar.activation(nrm_sb, psn, AF.Copy)
            nc.scalar.dma_start(
                out=bass.AP(tensor=nrmd, offset=0, ap=[[1, T]]), in_=nrm_sb)
            nrm2 = qp.tile([128, NT], F32, tag="nrm2")
            nc.scalar.dma_start(
                out=nrm2,
                in_=bass.AP(tensor=nrmd, offset=0, ap=[[1, 128], [128, NT]]))
            nrms = qp.tile([128, NT], F32, tag="nrms")
            nc.scalar.activation(nrms, nrm2, AF.Sqrt)
            nc.vector.reciprocal(rq, nrms)

        # ---- phase B: sims + scan per group sweep; tails drip into the
        # next sweep as per-chunk closures so the PE never stalls ----
        with tc.tile_pool(name="kbp", bufs=3) as kbp, \
             tc.tile_pool(name="evp", bufs=2) as evp, \
             tc.tile_pool(name="tailp", bufs=1) as tp, \
             tc.tile_pool(name="gathp", bufs=6) as gp, \
             tc.tile_pool(name="diagp", bufs=4) as dgp, \
             tc.tile_pool(name="gop", bufs=2) as gop, \
             tc.tile_pool(name="simps", bufs=2, space="PSUM") as sps, \
             tc.tile_pool(name="dps", bufs=2, space="PSUM") as dps:
            groups = [(0, 1), (2,), (3,)]
            pending = []          # deferred tail closures (DVE + PE work)
            for gi, tiles in enumerate(groups):
                for b in range(NB):
                    kb = kbp.tile([128, n_dk, cfg.block], F32, tag="kb",
                                  name="kb")
                    nc.sync.dma_start(out=kb, in_=kpk[b, :, :, :])
                    for i in tiles:
                        nch = cfg.block // 512
                        pss = sps.tile([128, cfg.block], F32, tag="sim",
                                       name="sim")
                        for ckp in range(n_dk):
                            qch = qT_sb[:, ckp, 128 * i:128 * (i + 1)]
                            for c2 in range(nch):
                                sl = slice(512 * c2, 512 * (c2 + 1))
                                nc.tensor.matmul(pss[:, sl], qch,
                                                 kb[:, ckp, sl],
                                                 start=(ckp == 0),
                                                 stop=(ckp == n_dk - 1),
                                                 skip_group_check=True)
                        nc.vector.max(candV[:, i, 8 * b:8 * b + 8], pss)
                        nc.vector.max_index(candP[:, i, 8 * b:8 * b + 8],
                                            candV[:, i, 8 * b:8 * b + 8],
                                            pss)
                    if b >= 1 and pending:
                        pending.pop(0)()
                    if b == NB // 2:
                        for i in tiles:
                            _gate_tile(tc, cfg, i, xb_sb, wg_sb, gate_sb,
                                       sps)
                # ---- end of sweep: index staging + gathers for all the
                # group's tiles first, then deferred softmax/PE closures ----
                sts = {}
                for i in tiles:
                    sts[i] = _tail_reduce(tc, cfg, i, candV, candP,
                                          shof_sb, stage, tp)
                for i in tiles:
                    _gather_start(tc, cfg, i, sts[i], vals, stage, tp, gp)
                per_tile = []
                for i in tiles:
                    cl = _soft_closures(tc, cfg, i, sts[i], candV, shof_sb,
                                        rq, tp)
                    cl += _out_closures(tc, cfg, i, sts[i], vals, identb,
                                        gate_sb, wo_sb, out, tp, gp, dgp,
                                        gop, dps)
                    per_tile.append(cl)
                for step in range(max(len(c) for c in per_tile)):
                    for cl in per_tile:
                        if step < len(cl):
                            pending.append(cl[step])
            for fn in pending:
                fn()


def _gate_tile(tc, cfg, i, xb_sb, wg_sb, gate_sb, sps):
    nc = tc.nc
    n_dm = cfg.d_model // 128
    psg = sps.tile([128, cfg.d_val], F32, tag="sim", name="psg")
    for d in range(n_dm):
        xch = xb_sb[:, d, 128 * i:128 * (i + 1)]
        for h in range(2):
            sl = slice(512 * h, 512 * (h + 1))
            nc.tensor.matmul(psg[:, sl], xch, wg_sb[:, d, sl],
                             start=(d == 0), stop=(d == n_dm - 1),
                             skip_group_check=True)
    # silu(x) = x * sigmoid(x) exactly, matching the reference
    nc.scalar.activation(gate_sb[:, i, :], psg, AF.Sigmoid)
    nc.vector.tensor_mul(gate_sb[:, i, :], gate_sb[:, i, :], psg)


def _tail_reduce(tc, cfg, i, candV, candP, shof_sb, stage, tp):
    """Exact top-32 membership + gather-index staging for tile i.  The
    softmax / value-extraction (DVE-heavy) is deferred to closures."""
    nc = tc.nc
    K, NCD = cfg.k, cfg.n_cand

    scr = tp.tile([128, NCD], F32, tag="scr", name="scr")
    nc.vector.tensor_copy(scr, candV[:, i, :])
    mx = tp.tile([128, K], F32, tag=f"mx_{i}", name=f"mx_{i}")
    for r in range(K // 8):
        nc.vector.max(mx[:, 8 * r:8 * r + 8], scr)
        if r < K // 8 - 1:
            nc.vector.match_replace(scr, mx[:, 8 * r:8 * r + 8], scr, NEG)
    t1 = mx[:, K - 1:K]

    mask = tp.tile([128, NCD], F32, tag="mask", name="mask")
    nc.vector.tensor_scalar(mask, candV[:, i, :], t1, None, ALU.is_ge)
    pfull = tp.tile([128, NCD], F32, tag=f"pfull_{i}", name=f"pfull_{i}")
    nc.vector.tensor_copy(pfull, candP[:, i, :])
    nc.vector.tensor_add(pfull, pfull, shof_sb)
    pfm = tp.tile([128, NCD], F32, tag="pfm", name="pfm")
    nc.vector.tensor_mul(pfm, pfull, mask)

    g32 = tp.tile([128, K], F32, tag=f"g32_{i}", name=f"g32_{i}")
    for r in range(K // 8):
        nc.vector.max(g32[:, 8 * r:8 * r + 8], pfm)
        if r < K // 8 - 1:
            nc.vector.match_replace(pfm, g32[:, 8 * r:8 * r + 8], pfm, 0.0)
    idx16 = tp.tile([128, K], I16, tag="idx16", name="idx16")
    nc.vector.tensor_scalar(idx16, g32, 1.0, None, ALU.subtract)

    # stage j-major to DRAM immediately -- the gather chain (stage -> wr ->
    # dma_gather) is latency-bound.  Tail DMAs ride the Activation hwdge
    # queue so they never head-of-line-block the kb stream on sync.
    # write pre-wrapped: stage[(p%16)*256 + (p//16) + 8*j] = idx16[p, j],
    # so the gather-index readback is contiguous 512B per partition
    nc.scalar.dma_start(
        out=bass.AP(tensor=stage, offset=i * K * 128,
                    ap=[[1, 8], [8 * K, 16], [8, K]]),
        in_=idx16)
    return {"mx": mx, "pfull": pfull, "g32": g32}


def _gather_start(tc, cfg, i, st, vals, stage, tp, gp):
    """Index readback + first two value-gather chunks for tile i."""
    nc = tc.nc
    K, dv = cfg.k, cfg.d_val

    wr = tp.tile([128, 8 * K], I16, tag=f"wr_{i}", name=f"wr_{i}")
    nc.scalar.dma_start(
        out=wr,
        in_=bass.AP(tensor=stage, offset=i * K * 128,
                    ap=[[0, 8], [8 * K, 16], [1, 8 * K]]))

    vgs = {}
    for jc in range(2):
        vg = gp.tile([128, cfg.gjc, dv], BF16, tag="vg", name="vg")
        nc.gpsimd.dma_gather(
            vg, vals, wr[:, 8 * cfg.gjc * jc:8 * cfg.gjc * (jc + 1)],
            num_idxs=128 * cfg.gjc, num_idxs_reg=128 * cfg.gjc,
            elem_size=dv, queue_num=jc % 2)
        vgs[jc] = vg
    st["wr"] = wr
    st["vgs"] = vgs


def _soft_closures(tc, cfg, i, st, candV, shof_sb, rq, tp):
    """Deferred DVE work for tile i: value extraction (8 is_eq ops per
    closure) and the softmax that produces w32."""
    nc = tc.nc
    K = cfg.k
    mx, pfull, g32 = st["mx"], st["pfull"], st["g32"]
    v32 = tp.tile([128, K], F32, tag=f"v32_{i}", name=f"v32_{i}")

    def eq_chunk(r):
        def run():
            eqscr = tp.tile([128, cfg.n_cand], F32, tag="eqscr",
                            name="eqscr")
            for j in range(8 * r, 8 * r + 8):
                nc.vector.scalar_tensor_tensor(
                    eqscr, pfull, g32[:, j:j + 1], candV[:, i, :],
                    op0=ALU.is_equal, op1=ALU.mult,
                    accum_out=v32[:, j:j + 1])
        return run

    def softmax():
        # softmax over rq * v32; mx[:,0] is the max logit pre-scale
        bexp = tp.tile([128, 1], F32, tag="bexp", name="bexp")
        nc.vector.scalar_tensor_tensor(bexp, mx[:, 0:1], -1.0,
                                       rq[:, i:i + 1],
                                       op0=ALU.mult, op1=ALU.mult)
        e32 = tp.tile([128, K], F32, tag="e32", name="e32")
        ssum = tp.tile([128, 1], F32, tag="ssum", name="ssum")
        nc.scalar.activation(e32, v32, AF.Exp, bias=bexp,
                             scale=rq[:, i:i + 1], accum_out=ssum)
        rs = tp.tile([128, 1], F32, tag="rs", name="rs")
        nc.vector.reciprocal(rs, ssum)
        w32 = tp.tile([128, K], F32, tag=f"w32_{i}", name=f"w32_{i}")
        nc.vector.tensor_scalar(w32, e32, rs, None, ALU.mult)
        st["w32"] = w32

    return [eq_chunk(r) for r in range(K // 8)] + [softmax]


def _out_closures(tc, cfg, i, st, vals, identb, gate_sb, wo_sb, out, tp,
                  gp, dgp, gop, dps):
    """Deferred PE work for tile i: per-chunk diag(w) matmuls into PSUM,
    then gate multiply + transpose + output projection."""
    nc = tc.nc
    dm, dv, K = cfg.d_model, cfg.d_val, cfg.k
    n_dv = dv // 128
    n_chunks = K // cfg.gjc
    wr, vgs = st["wr"], st["vgs"]
    psm_box = {}

    def chunk_closure(c):
        def run():
            w32 = st["w32"]
            if c == 0:
                psm_box["psm"] = dps.tile([128, dv], F32, tag="m512",
                                          name="psm")
            psm = psm_box["psm"]
            vg = vgs.pop(c)
            for jj in range(cfg.gjc):
                j = cfg.gjc * c + jj
                diag = dgp.tile([128, 128], BF16, tag="diag", name="diag")
                nc.scalar.activation(diag, identb, AF.Copy,
                                     scale=w32[:, j:j + 1])
                for h in range(2):
                    sl = slice(512 * h, 512 * (h + 1))
                    nc.tensor.matmul(psm[:, sl], diag, vg[:, jj, sl],
                                     start=(j == 0), stop=(j == K - 1),
                                     skip_group_check=True)
            # issue the next gather only after this chunk's matmuls are
            # registered as vg readers -- recycling a buffer before its
            # consumers are issued would let the DMA overwrite live data
            nxt = c + 2
            if nxt < n_chunks:
                vg2 = gp.tile([128, cfg.gjc, dv], BF16, tag="vg", name="vg")
                nc.gpsimd.dma_gather(
                    vg2, vals,
                    wr[:, 8 * cfg.gjc * nxt:8 * cfg.gjc * (nxt + 1)],
                    num_idxs=128 * cfg.gjc, num_idxs_reg=128 * cfg.gjc,
                    elem_size=dv, queue_num=nxt % 2)
                vgs[nxt] = vg2
        return run

    def finish():
        psm = psm_box["psm"]
        # y = mem * gate (bf16), reading mem straight out of PSUM
        y = gop.tile([128, dv], BF16, tag="y", name="y")
        nc.vector.tensor_mul(y, psm, gate_sb[:, i, :])

        yT = gop.tile([128, n_dv, 128], BF16, tag="yT", name="yT")
        nc.scalar.dma_start(out=yT, in_=y, transpose=True)
        out_sb = gop.tile([128, dm], F32, tag="outsb", name="outsb")
        pso = dps.tile([128, dm], F32, tag="m512", name="pso")
        for v in range(n_dv):
            for h in range(2):
                sl = slice(512 * h, 512 * (h + 1))
                nc.tensor.matmul(pso[:, sl], yT[:, v, :],
                                 wo_sb[:, v, sl],
                                 start=(v == 0), stop=(v == n_dv - 1),
                                 skip_group_check=True)
        nc.scalar.activation(out_sb, pso, AF.Copy)
        nc.scalar.dma_start(out=out[128 * i:128 * (i + 1), :], in_=out_sb)

    return [chunk_closure(c) for c in range(n_chunks)] + [finish]


# ---------------------------------------------------------------- host side

_CACHE = {}


def _prep(x, keys, values, w_q, w_gate, w_out, cfg):
    dm, dk, dv = cfg.d_model, cfg.d_key, cfg.d_val
    n_dm, n_dk, n_dv = dm // 128, dk // 128, dv // 128
    xf = np.ascontiguousarray(x.reshape(-1, dm)).astype(np.float32)

    norm = np.sqrt((keys.astype(np.float64) ** 2).sum(1, keepdims=True))
    knm = (keys / np.maximum(norm, 1e-12)).astype(np.float32)
    knT = np.ascontiguousarray(knm.T)             # [dk, n_mem]
    r = knT.reshape(n_dk, 128, cfg.n_blocks, cfg.block)
    kpk = np.ascontiguousarray(r.transpose(2, 1, 0, 3))

    wqT = np.ascontiguousarray(w_q.T)             # [dm, dk]
    wqp = np.ascontiguousarray(
        wqT.reshape(n_dm, 128, dk).transpose(1, 0, 2))

    wgT = np.ascontiguousarray(w_gate.T)          # [dm, dv]
    wgp = wgT.astype(ml_dtypes.bfloat16).reshape(n_dm, 128, dv)
    wgp = np.ascontiguousarray(wgp.transpose(1, 0, 2))
    woT = np.ascontiguousarray(w_out.T)           # [dv, dm]
    wop = woT.astype(ml_dtypes.bfloat16).reshape(n_dv, 128, dm)
    wop = np.ascontiguousarray(wop.transpose(1, 0, 2))

    shof = ((np.arange(cfg.n_cand, dtype=np.float32) // 8) * cfg.block
            + 1.0).astype(np.float32)
    common = {
        "kpk": kpk,
        "vals": np.ascontiguousarray(values).astype(ml_dtypes.bfloat16),
        "wqT": wqp,
        "wg": wgp,
        "wo": wop,
        "shof": shof,
    }
    in_maps = []
    for c in range(N_CORES):
        xc = xf[c * cfg.T:(c + 1) * cfg.T]        # [T, dm]
        xTc = np.ascontiguousarray(xc.T)          # [dm, T]
        xp = np.ascontiguousarray(
            xTc.reshape(n_dm, 128, cfg.T).transpose(1, 0, 2))
        m = dict(common)
        m["xT"] = xp
        in_maps.append(m)
    return in_maps


def kernel(x, keys, values, w_q, w_gate, w_out):
    cfg = FULL
    if "nc" not in _CACHE:
        _CACHE["nc"] = build(cfg)
    nc = _CACHE["nc"]
    x = np.asarray(x)
    in_maps = _prep(x, np.asarray(keys), np.asarray(values),
                    np.asarray(w_q), np.asarray(w_gate), np.asarray(w_out),
                    cfg)
    trace = os.environ.get("KERNEL_TRACE", "0") == "1"
    if trace:
        try:
            import ntff_shim
            ntff_shim.install()
        except Exception:
            pass
    res = run_bass_kernel_spmd(nc, in_maps, list(range(N_CORES)), trace=trace)
    if trace:
        _CACHE["exec_time_ns"] = res.exec_time_ns
    outs = [res.results[c]["out"] for c in range(N_CORES)]
    B, S, D = x.shape
    return np.concatenate(outs, axis=0).reshape(B, S, D)


# revision 17
# speedup vs baseline: 1.1916x; 1.0658x over previous
"""Trainium2 Bass kernel for nn_MemoryPlus (retrieval_knn).

Strategy (8 NeuronCores, data-parallel over the 4096 tokens, 512/core):
  sims = q @ k_norm^T runs in plain fp32 on the PE: top-32-of-32768
  selection gaps are ~4e-4 absolute vs bf16 noise ~1.5e-4, so 16-bit
  (and fp32r) matmuls mis-select members and blow the 2e-2 error
  budget; a single boundary swap costs ~4e-3 global rel err.  The
  top-8 per 1024-shard scan (max8 + find_index8) runs on fp32 values
  straight out of PSUM.  The 256 candidates reduce to an exact top-32
  (max8/match_replace), softmax runs on rq-scaled logits, value rows
  are fetched with gpsimd dma_gather (bf16) on two software-DGE
  queues, and accumulated on the PE via diag(w_j) matmuls into PSUM
  (the per-token softmax weight is folded into a diagonal stationary
  matrix built on the Scalar engine - no full-width scaled copies).
  gate and the output projection run in bf16; y^T for the output
  matmul comes from a DMA XBAR transpose.

  Scheduling: tiles sweep the 32 key blocks in three groups (0,1) /
  (2,) / (3,) (keys streamed three times).  A finished group's tail
  (top-32 + softmax + gather-index staging + first gathers) issues at
  high priority, but its PE work (diag matmuls + output matmul) is
  deferred and drip-fed into the next sweep's block loop, one gather
  chunk per block, so the in-order PE stream never stalls on gather
  latency.  Only tile 3's tail is exposed at the end.

Host-side work is layout only (transposes / normalization / dtype
packing of fixed weights+inputs).
"""

import os

import ml_dtypes
import numpy as np

import concourse.bass as bass
import concourse.tile as tile
from concourse import bacc, mybir
from concourse.bass_utils import run_bass_kernel_spmd
from concourse.masks import make_identity

F32 = mybir.dt.float32
BF16 = mybir.dt.bfloat16
I16 = mybir.dt.int16
U16 = mybir.dt.uint16
AF = mybir.ActivationFunctionType
ALU = mybir.AluOpType

N_CORES = 8
NEG = -1.0e30


class Cfg:
    def __init__(self, n_mem=32768, n_ttiles=4, d_model=1024, d_key=256,
                 d_val=1024, k=32, block=1024, gjc=4):
        self.n_mem = n_mem
        self.n_ttiles = n_ttiles          # token tiles of 128 per core
        self.T = 128 * n_ttiles           # tokens per core
        self.d_model = d_model
        self.d_key = d_key
        self.d_val = d_val
        self.k = k
        self.block = block                # mem block per k DMA (= shard)
        self.n_blocks = n_mem // block
        self.n_cand = 8 * self.n_blocks   # top-8 per shard
        self.gjc = gjc                    # value-gather j-chunk
        assert self.n_cand >= k and k % 8 == 0


FULL = Cfg()


def build(cfg: Cfg):
    nc = bacc.Bacc("TRN2", target_bir_lowering=False, debug=False,
                   num_devices=N_CORES, num_swdge_queues=2)
    dm, dk, dv, T = cfg.d_model, cfg.d_key, cfg.d_val, cfg.T
    n_dm, n_dk, n_dv = dm // 128, dk // 128, dv // 128

    xT = nc.dram_tensor("xT", [128, n_dm, T], F32, kind="ExternalInput").ap()
    wqT = nc.dram_tensor("wqT", [128, n_dm, dk], F32,
                         kind="ExternalInput").ap()
    kpk = nc.dram_tensor("kpk", [cfg.n_blocks, 128, n_dk, cfg.block],
                         F32, kind="ExternalInput").ap()
    wg = nc.dram_tensor("wg", [128, n_dm, dv], BF16,
                        kind="ExternalInput").ap()
    wo = nc.dram_tensor("wo", [128, n_dv, dm], BF16,
                        kind="ExternalInput").ap()
    vals = nc.dram_tensor("vals", [cfg.n_mem, dv], BF16,
                          kind="ExternalInput").ap()
    shof = nc.dram_tensor("shof", [cfg.n_cand], F32,
                          kind="ExternalInput").ap()
    out = nc.dram_tensor("out", [T, dm], F32, kind="ExternalOutput").ap()
    stage = nc.dram_tensor("stage", [cfg.n_ttiles * cfg.k * 128], I16)
    nrmd = nc.dram_tensor("nrmd", [T], F32)

    with tile.TileContext(nc) as tc:
        _kernel_body(tc, cfg, xT, wqT, kpk, wg, wo, vals, shof, out,
                     stage, nrmd)
    nc.compile()
    return nc


def _kernel_body(tc, cfg, xT, wqT, kpk, wg, wo, vals, shof, out,
                 stage, nrmd):
    nc = tc.nc
    dm, dk, dv, T, K = cfg.d_model, cfg.d_key, cfg.d_val, cfg.T, cfg.k
    n_dm, n_dk, n_dv = dm // 128, dk // 128, dv // 128
    NT = cfg.n_ttiles
    NCD = cfg.n_cand
    NB = cfg.n_blocks

    with tc.tile_pool(name="persist", bufs=1) as persist, \
         tc.tile_pool(name="kbp", bufs=3) as kbp:
        # prefetch the first two key blocks ahead of x/wq so sims can
        # start the moment qproj finishes
        kb_pre = {}
        for b in range(2):
            kb = kbp.tile([128, n_dk, cfg.block], F32, tag="kb", name="kb")
            nc.sync.dma_start(out=kb, in_=kpk[b, :, :, :])
            kb_pre[b] = kb

        ident = persist.tile([128, 128], F32)
        make_identity(nc, ident)
        identb = persist.tile([128, 128], BF16)
        nc.vector.tensor_copy(identb, ident)

        # chunked input DMAs so qproj starts on the first 256KB
        xT_sb = persist.tile([128, n_dm, T], F32)
        for d in range(n_dm):
            nc.sync.dma_start(out=xT_sb[:, d, :], in_=xT[:, d, :])
        xb_sb = persist.tile([128, n_dm, T], BF16)
        nc.scalar.activation(xb_sb, xT_sb, AF.Copy)
        wg_sb = persist.tile([128, n_dm, dv], BF16)
        nc.sync.dma_start(out=wg_sb, in_=wg)
        wo_sb = persist.tile([128, n_dv, dm], BF16)
        nc.scalar.dma_start(out=wo_sb, in_=wo)
        shof_sb = persist.tile([128, NCD], F32)
        nc.scalar.dma_start(
            out=shof_sb,
            in_=bass.AP(tensor=shof.tensor, offset=0, ap=[[0, 128], [1, NCD]]))

        qT_sb = persist.tile([128, n_dk, T], F32)
        rq = persist.tile([128, NT], F32)
        candV = persist.tile([128, NT, NCD], F32)
        candP = persist.tile([128, NT, NCD], U16)
        gate_sb = persist.tile([128, NT, dv], BF16)

        # ---- phase A: qT = wq^T x (fp32) and rq = 1/|q| ----
        with tc.tile_pool(name="qphase", bufs=1) as qp, \
             tc.tile_pool(name="qps", bufs=2, space="PSUM") as qps:
            wq_sb = qp.tile([128, n_dm, dk], F32, tag="wq")
            for d in range(n_dm):
                nc.sync.dma_start(out=wq_sb[:, d, :], in_=wqT[:, d, :])
            for ckp in range(n_dk):
                ps = qps.tile([128, T], F32, tag="qmm")
                for d in range(n_dm):
                    nc.tensor.matmul(ps,
                                     wq_sb[:, d, 128 * ckp:128 * (ckp + 1)],
                                     xT_sb[:, d, :],
                                     start=(d == 0), stop=(d == n_dm - 1))
                nc.scalar.activation(qT_sb[:, ckp, :], ps, AF.Copy)

            # |q|^2 per token via ones-matmul; DRAM round-trip to [128, NT]
            sq = qp.tile([128, n_dk, T], F32, tag="sq")
            nc.scalar.activation(sq, qT_sb, AF.Square)
            ones = qp.tile([128, 1], F32, tag="ones")
            nc.vector.memset(ones, 1.0)
            psn = qps.tile([1, T], F32, tag="qnrm")
            for ckp in range(n_dk):
                nc.tensor.matmul(psn, ones, sq[:, ckp, :],
                                 start=(ckp == 0), stop=(ckp == n_dk - 1))
            nrm_sb = qp.tile([1, T], F32, tag="nrm")
            nc.scalar.activation(nrm_sb, psn, AF.Copy)
            nc.scalar.dma_start(
                out=bass.AP(tensor=nrmd, offset=0, ap=[[1, T]]), in_=nrm_sb)
            nrm2 = qp.tile([128, NT], F32, tag="nrm2")
            nc.scalar.dma_start(
                out=nrm2,
                in_=bass.AP(tensor=nrmd, offset=0, ap=[[1, 128], [128, NT]]))
            nrms = qp.tile([128, NT], F32, tag="nrms")
            nc.scalar.activation(nrms, nrm2, AF.Sqrt)
            nc.vector.reciprocal(rq, nrms)

        # ---- phase B: sims + scan per group sweep; tails drip into the
        # next sweep as per-chunk closures so the PE never stalls ----
        with tc.tile_pool(name="tailp", bufs=1) as tp, \
             tc.tile_pool(name="gathp", bufs=6) as gp, \
             tc.tile_pool(name="diagp", bufs=4) as dgp, \
             tc.tile_pool(name="gop", bufs=2) as gop, \
             tc.tile_pool(name="simps", bufs=2, space="PSUM") as sps, \
             tc.tile_pool(name="dps", bufs=2, space="PSUM") as dps:
            groups = [(0, 1), (2,), (3,)]
            pending = []          # deferred tail closures (DVE + PE work)
            for gi, tiles in enumerate(groups):
                for b in range(NB):
                    if gi == 0 and b in kb_pre:
                        kb = kb_pre.pop(b)
                    else:
                        kb = kbp.tile([128, n_dk, cfg.block], F32,
                                      tag="kb", name="kb")
                        nc.sync.dma_start(out=kb, in_=kpk[b, :, :, :])
                    for i in tiles:
                        nch = cfg.block // 512
                        pss = sps.tile([128, cfg.block], F32, tag="sim",
                                       name="sim")
                        for ckp in range(n_dk):
                            qch = qT_sb[:, ckp, 128 * i:128 * (i + 1)]
                            for c2 in range(nch):
                                sl = slice(512 * c2, 512 * (c2 + 1))
                                nc.tensor.matmul(pss[:, sl], qch,
                                                 kb[:, ckp, sl],
                                                 start=(ckp == 0),
                                                 stop=(ckp == n_dk - 1),
                                                 skip_group_check=True)
                        nc.vector.max(candV[:, i, 8 * b:8 * b + 8], pss)
                        nc.vector.max_index(candP[:, i, 8 * b:8 * b + 8],
                                            candV[:, i, 8 * b:8 * b + 8],
                                            pss)
                    if b >= 1 and pending:
                        pending.pop(0)()
                    if b == NB // 2:
                        for i in tiles:
                            _gate_tile(tc, cfg, i, xb_sb, wg_sb, gate_sb,
                                       sps)
                # ---- end of sweep: index staging + gathers for all the
                # group's tiles first, then deferred softmax/PE closures ----
                sts = {}
                for i in tiles:
                    sts[i] = _tail_reduce(tc, cfg, i, candV, candP,
                                          shof_sb, stage, tp)
                for i in tiles:
                    _gather_start(tc, cfg, i, sts[i], vals, stage, tp, gp)
                per_tile = []
                for i in tiles:
                    cl = _soft_closures(tc, cfg, i, sts[i], candV, shof_sb,
                                        rq, tp)
                    cl += _out_closures(tc, cfg, i, sts[i], vals, identb,
                                        gate_sb, wo_sb, out, tp, gp, dgp,
                                        gop, dps)
                    per_tile.append(cl)
                for step in range(max(len(c) for c in per_tile)):
                    for cl in per_tile:
                        if step < len(cl):
                            pending.append(cl[step])
            for fn in pending:
                fn()


def _gate_tile(tc, cfg, i, xb_sb, wg_sb, gate_sb, sps):
    nc = tc.nc
    n_dm = cfg.d_model // 128
    psg = sps.tile([128, cfg.d_val], F32, tag="sim", name="psg")
    for d in range(n_dm):
        xch = xb_sb[:, d, 128 * i:128 * (i + 1)]
        for h in range(2):
            sl = slice(512 * h, 512 * (h + 1))
            nc.tensor.matmul(psg[:, sl], xch, wg_sb[:, d, sl],
                             start=(d == 0), stop=(d == n_dm - 1),
                             skip_group_check=True)
    # silu(x) = x * sigmoid(x) exactly, matching the reference
    nc.scalar.activation(gate_sb[:, i, :], psg, AF.Sigmoid)
    nc.vector.tensor_mul(gate_sb[:, i, :], gate_sb[:, i, :], psg)


def _tail_reduce(tc, cfg, i, candV, candP, shof_sb, stage, tp):
    """Exact top-32 membership + gather-index staging for tile i.  The
    softmax / value-extraction (DVE-heavy) is deferred to closures."""
    nc = tc.nc
    K, NCD = cfg.k, cfg.n_cand

    scr = tp.tile([128, NCD], F32, tag="scr", name="scr")
    nc.vector.tensor_copy(scr, candV[:, i, :])
    mx = tp.tile([128, K], F32, tag=f"mx_{i}", name=f"mx_{i}")
    for r in range(K // 8):
        nc.vector.max(mx[:, 8 * r:8 * r + 8], scr)
        if r < K // 8 - 1:
            nc.vector.match_replace(scr, mx[:, 8 * r:8 * r + 8], scr, NEG)
    t1 = mx[:, K - 1:K]

    mask = tp.tile([128, NCD], F32, tag="mask", name="mask")
    nc.vector.tensor_scalar(mask, candV[:, i, :], t1, None, ALU.is_ge)
    pfull = tp.tile([128, NCD], F32, tag=f"pfull_{i}", name=f"pfull_{i}")
    nc.vector.scalar_tensor_tensor(pfull, candP[:, i, :], 1.0, shof_sb,
                                   op0=ALU.mult, op1=ALU.add)
    pfm = tp.tile([128, NCD], F32, tag="pfm", name="pfm")
    nc.vector.tensor_mul(pfm, pfull, mask)

    g32 = tp.tile([128, K], F32, tag=f"g32_{i}", name=f"g32_{i}")
    for r in range(K // 8):
        nc.vector.max(g32[:, 8 * r:8 * r + 8], pfm)
        if r < K // 8 - 1:
            nc.vector.match_replace(pfm, g32[:, 8 * r:8 * r + 8], pfm, 0.0)
    idx16 = tp.tile([128, K], I16, tag="idx16", name="idx16")
    nc.vector.tensor_scalar(idx16, g32, 1.0, None, ALU.subtract)

    # stage j-major to DRAM immediately -- the gather chain (stage -> wr ->
    # dma_gather) is latency-bound.  The pre-wrapped scatter
    # stage[(p%16)*256 + (p//16) + 8*j] = idx16[p, j] interleaves tokens
    # mod 16, so it decomposes into 2-byte DMA rows (~40us of descriptor
    # processing when issued monolithically).  Split it by partition
    # group (t8 = p//16) into eight 2-dim DMAs alternated across the two
    # hwdge rings so the processing runs 2-way parallel off the kb
    # stream's critical path.
    for t8 in range(8):
        eng = nc.sync if t8 % 2 == 0 else nc.scalar
        eng.dma_start(
            out=bass.AP(tensor=stage, offset=i * K * 128 + t8,
                        ap=[[8 * K, 16], [8, K]]),
            in_=idx16[16 * t8:16 * (t8 + 1), :])
    return {"mx": mx, "pfull": pfull, "g32": g32}


def _gather_start(tc, cfg, i, st, vals, stage, tp, gp):
    """Index readback + first two value-gather chunks for tile i."""
    nc = tc.nc
    K, dv = cfg.k, cfg.d_val

    gjc = cfg.gjc
    wr = tp.tile([128, 8 * K], I16, tag=f"wr_{i}", name=f"wr_{i}")

    def read_chunk(c):
        eng = nc.sync if c % 2 == 0 else nc.scalar
        eng.dma_start(
            out=wr[:, 8 * gjc * c:8 * gjc * (c + 1)],
            in_=bass.AP(tensor=stage,
                        offset=i * K * 128 + 8 * gjc * c,
                        ap=[[0, 8], [8 * K, 16], [1, 8 * gjc]]))

    vgs = {}
    for jc in range(2):
        read_chunk(jc)
        vg = gp.tile([128, gjc, dv], BF16, tag="vg", name="vg")
        nc.gpsimd.dma_gather(
            vg, vals, wr[:, 8 * gjc * jc:8 * gjc * (jc + 1)],
            num_idxs=128 * gjc, num_idxs_reg=128 * gjc,
            elem_size=dv, queue_num=jc % 2)
        vgs[jc] = vg
    st["wr"] = wr
    st["vgs"] = vgs
    st["read_chunk"] = read_chunk


def _soft_closures(tc, cfg, i, st, candV, shof_sb, rq, tp):
    """Deferred DVE work for tile i: value extraction (8 is_eq ops per
    closure) and the softmax that produces w32."""
    nc = tc.nc
    K = cfg.k
    mx, pfull, g32 = st["mx"], st["pfull"], st["g32"]
    v32 = tp.tile([128, K], F32, tag=f"v32_{i}", name=f"v32_{i}")

    def eq_chunk(r):
        def run():
            eqscr = tp.tile([128, cfg.n_cand], F32, tag="eqscr",
                            name="eqscr")
            for j in range(8 * r, 8 * r + 8):
                nc.vector.scalar_tensor_tensor(
                    eqscr, pfull, g32[:, j:j + 1], candV[:, i, :],
                    op0=ALU.is_equal, op1=ALU.mult,
                    accum_out=v32[:, j:j + 1])
        return run

    def softmax():
        # softmax over rq * v32; mx[:,0] is the max logit pre-scale
        bexp = tp.tile([128, 1], F32, tag="bexp", name="bexp")
        nc.vector.scalar_tensor_tensor(bexp, mx[:, 0:1], -1.0,
                                       rq[:, i:i + 1],
                                       op0=ALU.mult, op1=ALU.mult)
        e32 = tp.tile([128, K], F32, tag="e32", name="e32")
        ssum = tp.tile([128, 1], F32, tag="ssum", name="ssum")
        nc.scalar.activation(e32, v32, AF.Exp, bias=bexp,
                             scale=rq[:, i:i + 1], accum_out=ssum)
        rs = tp.tile([128, 1], F32, tag="rs", name="rs")
        nc.vector.reciprocal(rs, ssum)
        w32 = tp.tile([128, K], F32, tag=f"w32_{i}", name=f"w32_{i}")
        nc.vector.tensor_scalar(w32, e32, rs, None, ALU.mult)
        st["w32"] = w32

    return [eq_chunk(r) for r in range(K // 8)] + [softmax]


def _out_closures(tc, cfg, i, st, vals, identb, gate_sb, wo_sb, out, tp,
                  gp, dgp, gop, dps):
    """Deferred PE work for tile i: per-chunk diag(w) matmuls into PSUM,
    then gate multiply + transpose + output projection."""
    nc = tc.nc
    dm, dv, K = cfg.d_model, cfg.d_val, cfg.k
    n_dv = dv // 128
    n_chunks = K // cfg.gjc
    wr, vgs = st["wr"], st["vgs"]
    psm_box = {}

    def chunk_closure(c):
        def run():
            w32 = st["w32"]
            if c == 0:
                psm_box["psm"] = dps.tile([128, dv], F32, tag="m512",
                                          name="psm")
            psm = psm_box["psm"]
            vg = vgs.pop(c)
            for jj in range(cfg.gjc):
                j = cfg.gjc * c + jj
                diag = dgp.tile([128, 128], BF16, tag="diag", name="diag")
                nc.scalar.activation(diag, identb, AF.Copy,
                                     scale=w32[:, j:j + 1])
                for h in range(2):
                    sl = slice(512 * h, 512 * (h + 1))
                    nc.tensor.matmul(psm[:, sl], diag, vg[:, jj, sl],
                                     start=(j == 0), stop=(j == K - 1),
                                     skip_group_check=True)
            # issue the next gather only after this chunk's matmuls are
            # registered as vg readers -- recycling a buffer before its
            # consumers are issued would let the DMA overwrite live data
            nxt = c + 2
            if nxt < n_chunks:
                st["read_chunk"](nxt)
                vg2 = gp.tile([128, cfg.gjc, dv], BF16, tag="vg", name="vg")
                nc.gpsimd.dma_gather(
                    vg2, vals,
                    wr[:, 8 * cfg.gjc * nxt:8 * cfg.gjc * (nxt + 1)],
                    num_idxs=128 * cfg.gjc, num_idxs_reg=128 * cfg.gjc,
                    elem_size=dv, queue_num=nxt % 2)
                vgs[nxt] = vg2
        return run

    def finish():
        psm = psm_box["psm"]
        # y = mem * gate (bf16), reading mem straight out of PSUM
        y = gop.tile([128, dv], BF16, tag="y", name="y")
        nc.vector.tensor_mul(y, psm, gate_sb[:, i, :])

        yT = gop.tile([128, n_dv, 128], BF16, tag="yT", name="yT")
        nc.scalar.dma_start(out=yT, in_=y, transpose=True)
        out_sb = gop.tile([128, dm], F32, tag="outsb", name="outsb")
        pso = dps.tile([128, dm], F32, tag="m512", name="pso")
        for v in range(n_dv):
            for h in range(2):
                sl = slice(512 * h, 512 * (h + 1))
                nc.tensor.matmul(pso[:, sl], yT[:, v, :],
                                 wo_sb[:, v, sl],
                                 start=(v == 0), stop=(v == n_dv - 1),
                                 skip_group_check=True)
        nc.scalar.activation(out_sb, pso, AF.Copy)
        nc.scalar.dma_start(out=out[128 * i:128 * (i + 1), :], in_=out_sb)

    return [chunk_closure(c) for c in range(n_chunks)] + [finish]


# ---------------------------------------------------------------- host side

_CACHE = {}


def _prep(x, keys, values, w_q, w_gate, w_out, cfg):
    dm, dk, dv = cfg.d_model, cfg.d_key, cfg.d_val
    n_dm, n_dk, n_dv = dm // 128, dk // 128, dv // 128
    xf = np.ascontiguousarray(x.reshape(-1, dm)).astype(np.float32)

    norm = np.sqrt((keys.astype(np.float64) ** 2).sum(1, keepdims=True))
    knm = (keys / np.maximum(norm, 1e-12)).astype(np.float32)
    knT = np.ascontiguousarray(knm.T)             # [dk, n_mem]
    r = knT.reshape(n_dk, 128, cfg.n_blocks, cfg.block)
    kpk = np.ascontiguousarray(r.transpose(2, 1, 0, 3))

    wqT = np.ascontiguousarray(w_q.T)             # [dm, dk]
    wqp = np.ascontiguousarray(
        wqT.reshape(n_dm, 128, dk).transpose(1, 0, 2))

    wgT = np.ascontiguousarray(w_gate.T)          # [dm, dv]
    wgp = wgT.astype(ml_dtypes.bfloat16).reshape(n_dm, 128, dv)
    wgp = np.ascontiguousarray(wgp.transpose(1, 0, 2))
    woT = np.ascontiguousarray(w_out.T)           # [dv, dm]
    wop = woT.astype(ml_dtypes.bfloat16).reshape(n_dv, 128, dm)
    wop = np.ascontiguousarray(wop.transpose(1, 0, 2))

    shof = ((np.arange(cfg.n_cand, dtype=np.float32) // 8) * cfg.block
            + 1.0).astype(np.float32)
    common = {
        "kpk": kpk,
        "vals": np.ascontiguousarray(values).astype(ml_dtypes.bfloat16),
        "wqT": wqp,
        "wg": wgp,
        "wo": wop,
        "shof": shof,
    }
    in_maps = []
    for c in range(N_CORES):
        xc = xf[c * cfg.T:(c + 1) * cfg.T]        # [T, dm]
        xTc = np.ascontiguousarray(xc.T)          # [dm, T]
        xp = np.ascontiguousarray(
            xTc.reshape(n_dm, 128, cfg.T).transpose(1, 0, 2))
        m = dict(common)
        m["xT"] = xp
        in_maps.append(m)
    return in_maps


def kernel(x, keys, values, w_q, w_gate, w_out):
    cfg = FULL
    if "nc" not in _CACHE:
        _CACHE["nc"] = build(cfg)
    nc = _CACHE["nc"]
    x = np.asarray(x)
    in_maps = _prep(x, np.asarray(keys), np.asarray(values),
                    np.asarray(w_q), np.asarray(w_gate), np.asarray(w_out),
                    cfg)
    trace = os.environ.get("KERNEL_TRACE", "0") == "1"
    if trace:
        try:
            import ntff_shim
            ntff_shim.install()
        except Exception:
            pass
    res = run_bass_kernel_spmd(nc, in_maps, list(range(N_CORES)), trace=trace)
    if trace:
        _CACHE["exec_time_ns"] = res.exec_time_ns
    outs = [res.results[c]["out"] for c in range(N_CORES)]
    B, S, D = x.shape
    return np.concatenate(outs, axis=0).reshape(B, S, D)


# revision 20
# speedup vs baseline: 1.2086x; 1.0143x over previous
"""Trainium2 Bass kernel for nn_MemoryPlus (retrieval_knn).

Strategy (8 NeuronCores, data-parallel over the 4096 tokens, 512/core):
  sims = q @ k_norm^T runs in plain fp32 on the PE: top-32-of-32768
  selection gaps are ~4e-4 absolute vs bf16 noise ~1.5e-4, so 16-bit
  (and fp32r) matmuls mis-select members and blow the 2e-2 error
  budget; a single boundary swap costs ~4e-3 global rel err.  The
  top-8 per 1024-shard scan (max8 + find_index8) runs on fp32 values
  straight out of PSUM.  The 256 candidates reduce to an exact top-32
  (max8/match_replace), softmax runs on rq-scaled logits, value rows
  are fetched with gpsimd dma_gather (bf16) on two software-DGE
  queues, and accumulated on the PE via diag(w_j) matmuls into PSUM
  (the per-token softmax weight is folded into a diagonal stationary
  matrix built on the Scalar engine - no full-width scaled copies).
  gate and the output projection run in bf16; y^T for the output
  matmul comes from a DMA XBAR transpose.

  Scheduling: tiles sweep the 32 key blocks in three groups (0,1) /
  (2,) / (3,) (keys streamed three times).  A finished group's tail
  (top-32 + softmax + gather-index staging + first gathers) issues at
  high priority, but its PE work (diag matmuls + output matmul) is
  deferred and drip-fed into the next sweep's block loop, one gather
  chunk per block, so the in-order PE stream never stalls on gather
  latency.  Only tile 3's tail is exposed at the end.

Host-side work is layout only (transposes / normalization / dtype
packing of fixed weights+inputs).
"""

import os

import ml_dtypes
import numpy as np

import concourse.bass as bass
import concourse.tile as tile
from concourse import bacc, mybir
from concourse.bass_utils import run_bass_kernel_spmd
from concourse.masks import make_identity

F32 = mybir.dt.float32
BF16 = mybir.dt.bfloat16
I16 = mybir.dt.int16
U16 = mybir.dt.uint16
AF = mybir.ActivationFunctionType
ALU = mybir.AluOpType

N_CORES = 8
NEG = -1.0e30


class Cfg:
    def __init__(self, n_mem=32768, n_ttiles=4, d_model=1024, d_key=256,
                 d_val=1024, k=32, block=1024, gjc=4):
        self.n_mem = n_mem
        self.n_ttiles = n_ttiles          # token tiles of 128 per core
        self.T = 128 * n_ttiles           # tokens per core
        self.d_model = d_model
        self.d_key = d_key
        self.d_val = d_val
        self.k = k
        self.block = block                # mem block per k DMA (= shard)
        self.n_blocks = n_mem // block
        self.n_cand = 8 * self.n_blocks   # top-8 per shard
        self.gjc = gjc                    # value-gather j-chunk
        assert self.n_cand >= k and k % 8 == 0


FULL = Cfg()


def build(cfg: Cfg):
    nc = bacc.Bacc("TRN2", target_bir_lowering=False, debug=False,
                   num_devices=N_CORES, num_swdge_queues=2)
    dm, dk, dv, T = cfg.d_model, cfg.d_key, cfg.d_val, cfg.T
    n_dm, n_dk, n_dv = dm // 128, dk // 128, dv // 128

    xT = nc.dram_tensor("xT", [128, n_dm, T], F32, kind="ExternalInput").ap()
    wqT = nc.dram_tensor("wqT", [128, n_dm, dk], F32,
                         kind="ExternalInput").ap()
    kpk = nc.dram_tensor("kpk", [cfg.n_blocks, 128, n_dk, cfg.block],
                         F32, kind="ExternalInput").ap()
    wg = nc.dram_tensor("wg", [128, n_dm, dv], BF16,
                        kind="ExternalInput").ap()
    wo = nc.dram_tensor("wo", [128, n_dv, dm], BF16,
                        kind="ExternalInput").ap()
    vals = nc.dram_tensor("vals", [cfg.n_mem, dv], BF16,
                          kind="ExternalInput").ap()
    shof = nc.dram_tensor("shof", [cfg.n_cand], F32,
                          kind="ExternalInput").ap()
    out = nc.dram_tensor("out", [T, dm], F32, kind="ExternalOutput").ap()
    stage = nc.dram_tensor("stage", [cfg.n_ttiles * cfg.k * 128], I16)
    nrmd = nc.dram_tensor("nrmd", [T], F32)

    with tile.TileContext(nc) as tc:
        _kernel_body(tc, cfg, xT, wqT, kpk, wg, wo, vals, shof, out,
                     stage, nrmd)
    nc.compile()
    return nc


def _kernel_body(tc, cfg, xT, wqT, kpk, wg, wo, vals, shof, out,
                 stage, nrmd):
    nc = tc.nc
    dm, dk, dv, T, K = cfg.d_model, cfg.d_key, cfg.d_val, cfg.T, cfg.k
    n_dm, n_dk, n_dv = dm // 128, dk // 128, dv // 128
    NT = cfg.n_ttiles
    NCD = cfg.n_cand
    NB = cfg.n_blocks

    with tc.tile_pool(name="persist", bufs=1) as persist, \
         tc.tile_pool(name="kbp", bufs=3) as kbp:
        # prefetch the first two key blocks ahead of x/wq so sims can
        # start the moment qproj finishes
        kb_pre = {}
        for b in range(2):
            kb = kbp.tile([128, n_dk, cfg.block], F32, tag="kb", name="kb")
            nc.sync.dma_start(out=kb, in_=kpk[b, :, :, :])
            kb_pre[b] = kb

        ident = persist.tile([128, 128], F32)
        make_identity(nc, ident)
        identb = persist.tile([128, 128], BF16)
        nc.vector.tensor_copy(identb, ident)

        qT_sb = persist.tile([128, n_dk, T], F32)
        rq = persist.tile([128, NT], F32)
        candV = persist.tile([128, NT, NCD], F32)
        candP = persist.tile([128, NT, NCD], U16)
        gate_sb = persist.tile([128, NT, dv], BF16)
        xT_sb = persist.tile([128, n_dm, T], F32)
        xb_sb = persist.tile([128, n_dm, T], BF16)
        wg_sb = persist.tile([128, n_dm, dv], BF16)
        wo_sb = persist.tile([128, n_dv, dm], BF16)
        shof_sb = persist.tile([128, NCD], F32)

        # ---- phase A: qT = wq^T x (fp32) and rq = 1/|q| ----
        with tc.tile_pool(name="qphase", bufs=1) as qp, \
             tc.tile_pool(name="qps", bufs=2, space="PSUM") as qps:
            # interleave wq/x chunk loads so the first accumulation step
            # has its operands as early as possible
            wq_sb = qp.tile([128, n_dm, dk], F32, tag="wq")
            for d in range(n_dm):
                nc.sync.dma_start(out=wq_sb[:, d, :], in_=wqT[:, d, :])
                nc.sync.dma_start(out=xT_sb[:, d, :], in_=xT[:, d, :])
            for ckp in range(n_dk):
                ps = qps.tile([128, T], F32, tag="qmm")
                for d in range(n_dm):
                    nc.tensor.matmul(ps,
                                     wq_sb[:, d, 128 * ckp:128 * (ckp + 1)],
                                     xT_sb[:, d, :],
                                     start=(d == 0), stop=(d == n_dm - 1))
                nc.scalar.activation(qT_sb[:, ckp, :], ps, AF.Copy)

            # |q|^2 per token via ones-matmul; DRAM round-trip to [128, NT]
            sq = qp.tile([128, n_dk, T], F32, tag="sq")
            nc.scalar.activation(sq, qT_sb, AF.Square)
            ones = qp.tile([128, 1], F32, tag="ones")
            nc.vector.memset(ones, 1.0)
            psn = qps.tile([1, T], F32, tag="qnrm")
            for ckp in range(n_dk):
                nc.tensor.matmul(psn, ones, sq[:, ckp, :],
                                 start=(ckp == 0), stop=(ckp == n_dk - 1))
            nrm_sb = qp.tile([1, T], F32, tag="nrm")
            nc.scalar.activation(nrm_sb, psn, AF.Copy)
            nc.scalar.dma_start(
                out=bass.AP(tensor=nrmd, offset=0, ap=[[1, T]]), in_=nrm_sb)
            nrm2 = qp.tile([128, NT], F32, tag="nrm2")
            nc.scalar.dma_start(
                out=nrm2,
                in_=bass.AP(tensor=nrmd, offset=0, ap=[[1, 128], [128, NT]]))
            nrms = qp.tile([128, NT], F32, tag="nrms")
            nc.scalar.activation(nrms, nrm2, AF.Sqrt)
            nc.vector.reciprocal(rq, nrms)

            # deferred loads/conversions -- none of these gate the sims
            for d in range(n_dm):
                nc.scalar.activation(xb_sb[:, d, :], xT_sb[:, d, :],
                                     AF.Copy)
            nc.sync.dma_start(out=wg_sb, in_=wg)
            nc.scalar.dma_start(out=wo_sb, in_=wo)
            nc.scalar.dma_start(
                out=shof_sb,
                in_=bass.AP(tensor=shof.tensor, offset=0,
                            ap=[[0, 128], [1, NCD]]))

        # ---- phase B: sims + scan per group sweep; tails drip into the
        # next sweep as per-chunk closures so the PE never stalls ----
        with tc.tile_pool(name="tailp", bufs=1) as tp, \
             tc.tile_pool(name="gathp", bufs=6) as gp, \
             tc.tile_pool(name="diagp", bufs=4) as dgp, \
             tc.tile_pool(name="gop", bufs=2) as gop, \
             tc.tile_pool(name="simps", bufs=2, space="PSUM") as sps, \
             tc.tile_pool(name="dps", bufs=2, space="PSUM") as dps:
            groups = [(0, 1), (2,), (3,)]
            pending = []          # deferred tail closures (DVE + PE work)
            for gi, tiles in enumerate(groups):
                for b in range(NB):
                    if gi == 0 and b in kb_pre:
                        kb = kb_pre.pop(b)
                    else:
                        kb = kbp.tile([128, n_dk, cfg.block], F32,
                                      tag="kb", name="kb")
                        nc.sync.dma_start(out=kb, in_=kpk[b, :, :, :])
                    for i in tiles:
                        nch = cfg.block // 512
                        pss = sps.tile([128, cfg.block], F32, tag="sim",
                                       name="sim")
                        for ckp in range(n_dk):
                            qch = qT_sb[:, ckp, 128 * i:128 * (i + 1)]
                            for c2 in range(nch):
                                sl = slice(512 * c2, 512 * (c2 + 1))
                                nc.tensor.matmul(pss[:, sl], qch,
                                                 kb[:, ckp, sl],
                                                 start=(ckp == 0),
                                                 stop=(ckp == n_dk - 1),
                                                 skip_group_check=True)
                        nc.vector.max(candV[:, i, 8 * b:8 * b + 8], pss)
                        nc.vector.max_index(candP[:, i, 8 * b:8 * b + 8],
                                            candV[:, i, 8 * b:8 * b + 8],
                                            pss)
                    if b >= 1 and pending:
                        pending.pop(0)()
                    if b == NB // 2:
                        for i in tiles:
                            _gate_tile(tc, cfg, i, xb_sb, wg_sb, gate_sb,
                                       sps)
                # ---- end of sweep: index staging + gathers for all the
                # group's tiles first, then deferred softmax/PE closures ----
                sts = {}
                for i in tiles:
                    sts[i] = _tail_reduce(tc, cfg, i, candV, candP,
                                          shof_sb, stage, tp)
                for i in tiles:
                    _gather_start(tc, cfg, i, sts[i], vals, stage, tp, gp)
                per_tile = []
                for i in tiles:
                    cl = _soft_closures(tc, cfg, i, sts[i], candV, shof_sb,
                                        rq, tp)
                    cl += _out_closures(tc, cfg, i, sts[i], vals, identb,
                                        gate_sb, wo_sb, out, tp, gp, dgp,
                                        gop, dps)
                    per_tile.append(cl)
                for step in range(max(len(c) for c in per_tile)):
                    for cl in per_tile:
                        if step < len(cl):
                            pending.append(cl[step])
            for fn in pending:
                fn()


def _gate_tile(tc, cfg, i, xb_sb, wg_sb, gate_sb, sps):
    nc = tc.nc
    n_dm = cfg.d_model // 128
    psg = sps.tile([128, cfg.d_val], F32, tag="sim", name="psg")
    for d in range(n_dm):
        xch = xb_sb[:, d, 128 * i:128 * (i + 1)]
        for h in range(2):
            sl = slice(512 * h, 512 * (h + 1))
            nc.tensor.matmul(psg[:, sl], xch, wg_sb[:, d, sl],
                             start=(d == 0), stop=(d == n_dm - 1),
                             skip_group_check=True)
    # silu(x) = x * sigmoid(x) exactly, matching the reference
    nc.scalar.activation(gate_sb[:, i, :], psg, AF.Sigmoid)
    nc.vector.tensor_mul(gate_sb[:, i, :], gate_sb[:, i, :], psg)


def _tail_reduce(tc, cfg, i, candV, candP, shof_sb, stage, tp):
    """Exact top-32 membership + gather-index staging for tile i.  The
    softmax / value-extraction (DVE-heavy) is deferred to closures."""
    nc = tc.nc
    K, NCD = cfg.k, cfg.n_cand

    scr = tp.tile([128, NCD], F32, tag="scr", name="scr")
    nc.vector.tensor_copy(scr, candV[:, i, :])
    mx = tp.tile([128, K], F32, tag=f"mx_{i}", name=f"mx_{i}")
    for r in range(K // 8):
        nc.vector.max(mx[:, 8 * r:8 * r + 8], scr)
        if r < K // 8 - 1:
            nc.vector.match_replace(scr, mx[:, 8 * r:8 * r + 8], scr, NEG)
    t1 = mx[:, K - 1:K]

    mask = tp.tile([128, NCD], F32, tag="mask", name="mask")
    nc.vector.tensor_scalar(mask, candV[:, i, :], t1, None, ALU.is_ge)
    pfull = tp.tile([128, NCD], F32, tag=f"pfull_{i}", name=f"pfull_{i}")
    nc.vector.scalar_tensor_tensor(pfull, candP[:, i, :], 1.0, shof_sb,
                                   op0=ALU.mult, op1=ALU.add)
    pfm = tp.tile([128, NCD], F32, tag="pfm", name="pfm")
    nc.vector.tensor_mul(pfm, pfull, mask)

    g32 = tp.tile([128, K], F32, tag=f"g32_{i}", name=f"g32_{i}")
    for r in range(K // 8):
        nc.vector.max(g32[:, 8 * r:8 * r + 8], pfm)
        if r < K // 8 - 1:
            nc.vector.match_replace(pfm, g32[:, 8 * r:8 * r + 8], pfm, 0.0)
    idx16 = tp.tile([128, K], I16, tag=f"idx16_{i}", name=f"idx16_{i}")
    nc.vector.tensor_scalar(idx16, g32, 1.0, None, ALU.subtract)

    # stage j-major to DRAM immediately -- the gather chain (stage -> wr ->
    # dma_gather) is latency-bound.  The pre-wrapped scatter
    # stage[(p%16)*256 + (p//16) + 8*j] = idx16[p, j] interleaves tokens
    # mod 16, so it decomposes into 2-byte DMA rows (~40us of descriptor
    # processing when issued monolithically).  Split it by partition
    # group (t8 = p//16) into eight 2-dim DMAs alternated across the two
    # hwdge rings so the processing runs 2-way parallel off the kb
    # stream's critical path.
    for t8 in range(8):
        eng = nc.sync if t8 % 2 == 0 else nc.scalar
        eng.dma_start(
            out=bass.AP(tensor=stage, offset=i * K * 128 + t8,
                        ap=[[8 * K, 16], [8, K]]),
            in_=idx16[16 * t8:16 * (t8 + 1), :])
    return {"mx": mx, "pfull": pfull, "g32": g32}


def _gather_start(tc, cfg, i, st, vals, stage, tp, gp):
    """Index readback + first two value-gather chunks for tile i."""
    nc = tc.nc
    K, dv = cfg.k, cfg.d_val

    gjc = cfg.gjc
    wr = tp.tile([128, 8 * K], I16, tag=f"wr_{i}", name=f"wr_{i}")

    def read_chunk(c):
        eng = nc.sync if c % 2 == 0 else nc.scalar
        eng.dma_start(
            out=wr[:, 8 * gjc * c:8 * gjc * (c + 1)],
            in_=bass.AP(tensor=stage,
                        offset=i * K * 128 + 8 * gjc * c,
                        ap=[[0, 8], [8 * K, 16], [1, 8 * gjc]]))

    vgs = {}
    for jc in range(2):
        read_chunk(jc)
        vg = gp.tile([128, gjc, dv], BF16, tag="vg", name="vg")
        nc.gpsimd.dma_gather(
            vg, vals, wr[:, 8 * gjc * jc:8 * gjc * (jc + 1)],
            num_idxs=128 * gjc, num_idxs_reg=128 * gjc,
            elem_size=dv, queue_num=jc % 2)
        vgs[jc] = vg
    st["wr"] = wr
    st["vgs"] = vgs
    st["read_chunk"] = read_chunk


def _soft_closures(tc, cfg, i, st, candV, shof_sb, rq, tp):
    """Deferred DVE work for tile i: value extraction (8 is_eq ops per
    closure) and the softmax that produces w32."""
    nc = tc.nc
    K = cfg.k
    mx, pfull, g32 = st["mx"], st["pfull"], st["g32"]
    v32 = tp.tile([128, K], F32, tag=f"v32_{i}", name=f"v32_{i}")

    def eq_chunk(r):
        def run():
            eqscr = tp.tile([128, cfg.n_cand], F32, tag="eqscr",
                            name="eqscr")
            for j in range(8 * r, 8 * r + 8):
                nc.vector.scalar_tensor_tensor(
                    eqscr, pfull, g32[:, j:j + 1], candV[:, i, :],
                    op0=ALU.is_equal, op1=ALU.mult,
                    accum_out=v32[:, j:j + 1])
        return run

    def softmax():
        # softmax over rq * v32; mx[:,0] is the max logit pre-scale
        bexp = tp.tile([128, 1], F32, tag="bexp", name="bexp")
        nc.vector.scalar_tensor_tensor(bexp, mx[:, 0:1], -1.0,
                                       rq[:, i:i + 1],
                                       op0=ALU.mult, op1=ALU.mult)
        e32 = tp.tile([128, K], F32, tag="e32", name="e32")
        ssum = tp.tile([128, 1], F32, tag="ssum", name="ssum")
        nc.scalar.activation(e32, v32, AF.Exp, bias=bexp,
                             scale=rq[:, i:i + 1], accum_out=ssum)
        rs = tp.tile([128, 1], F32, tag="rs", name="rs")
        nc.vector.reciprocal(rs, ssum)
        w32 = tp.tile([128, K], F32, tag=f"w32_{i}", name=f"w32_{i}")
        nc.vector.tensor_scalar(w32, e32, rs, None, ALU.mult)
        st["w32"] = w32

    return [eq_chunk(r) for r in range(K // 8)] + [softmax]


def _out_closures(tc, cfg, i, st, vals, identb, gate_sb, wo_sb, out, tp,
                  gp, dgp, gop, dps):
    """Deferred PE work for tile i: per-chunk diag(w) matmuls into PSUM,
    then gate multiply + transpose + output projection."""
    nc = tc.nc
    dm, dv, K = cfg.d_model, cfg.d_val, cfg.k
    n_dv = dv // 128
    n_chunks = K // cfg.gjc
    wr, vgs = st["wr"], st["vgs"]
    psm_box = {}

    def chunk_closure(c):
        def run():
            w32 = st["w32"]
            if c == 0:
                psm_box["psm"] = dps.tile([128, dv], F32, tag="m512",
                                          name="psm")
            psm = psm_box["psm"]
            vg = vgs.pop(c)
            for jj in range(cfg.gjc):
                j = cfg.gjc * c + jj
                diag = dgp.tile([128, 128], BF16, tag="diag", name="diag")
                nc.scalar.activation(diag, identb, AF.Copy,
                                     scale=w32[:, j:j + 1])
                for h in range(2):
                    sl = slice(512 * h, 512 * (h + 1))
                    nc.tensor.matmul(psm[:, sl], diag, vg[:, jj, sl],
                                     start=(j == 0), stop=(j == K - 1),
                                     skip_group_check=True)
            # issue the next gather only after this chunk's matmuls are
            # registered as vg readers -- recycling a buffer before its
            # consumers are issued would let the DMA overwrite live data
            nxt = c + 2
            if nxt < n_chunks:
                st["read_chunk"](nxt)
                vg2 = gp.tile([128, cfg.gjc, dv], BF16, tag="vg", name="vg")
                nc.gpsimd.dma_gather(
                    vg2, vals,
                    wr[:, 8 * cfg.gjc * nxt:8 * cfg.gjc * (nxt + 1)],
                    num_idxs=128 * cfg.gjc, num_idxs_reg=128 * cfg.gjc,
                    elem_size=dv, queue_num=nxt % 2)
                vgs[nxt] = vg2
        return run

    def finish():
        psm = psm_box["psm"]
        # y = mem * gate (bf16), reading mem straight out of PSUM
        y = gop.tile([128, dv], BF16, tag="y", name="y")
        nc.vector.tensor_mul(y, psm, gate_sb[:, i, :])

        yT = gop.tile([128, n_dv, 128], BF16, tag="yT", name="yT")
        nc.scalar.dma_start(out=yT, in_=y, transpose=True)
        out_sb = gop.tile([128, dm], F32, tag="outsb", name="outsb")
        pso = dps.tile([128, dm], F32, tag="m512", name="pso")
        for v in range(n_dv):
            for h in range(2):
                sl = slice(512 * h, 512 * (h + 1))
                nc.tensor.matmul(pso[:, sl], yT[:, v, :],
                                 wo_sb[:, v, sl],
                                 start=(v == 0), stop=(v == n_dv - 1),
                                 skip_group_check=True)
        nc.scalar.activation(out_sb, pso, AF.Copy)
        nc.scalar.dma_start(out=out[128 * i:128 * (i + 1), :], in_=out_sb)

    return [chunk_closure(c) for c in range(n_chunks)] + [finish]


# ---------------------------------------------------------------- host side

_CACHE = {}


def _prep(x, keys, values, w_q, w_gate, w_out, cfg):
    dm, dk, dv = cfg.d_model, cfg.d_key, cfg.d_val
    n_dm, n_dk, n_dv = dm // 128, dk // 128, dv // 128
    xf = np.ascontiguousarray(x.reshape(-1, dm)).astype(np.float32)

    norm = np.sqrt((keys.astype(np.float64) ** 2).sum(1, keepdims=True))
    knm = (keys / np.maximum(norm, 1e-12)).astype(np.float32)
    knT = np.ascontiguousarray(knm.T)             # [dk, n_mem]
    r = knT.reshape(n_dk, 128, cfg.n_blocks, cfg.block)
    kpk = np.ascontiguousarray(r.transpose(2, 1, 0, 3))

    wqT = np.ascontiguousarray(w_q.T)             # [dm, dk]
    wqp = np.ascontiguousarray(
        wqT.reshape(n_dm, 128, dk).transpose(1, 0, 2))

    wgT = np.ascontiguousarray(w_gate.T)          # [dm, dv]
    wgp = wgT.astype(ml_dtypes.bfloat16).reshape(n_dm, 128, dv)
    wgp = np.ascontiguousarray(wgp.transpose(1, 0, 2))
    woT = np.ascontiguousarray(w_out.T)           # [dv, dm]
    wop = woT.astype(ml_dtypes.bfloat16).reshape(n_dv, 128, dm)
    wop = np.ascontiguousarray(wop.transpose(1, 0, 2))

    shof = ((np.arange(cfg.n_cand, dtype=np.float32) // 8) * cfg.block
            + 1.0).astype(np.float32)
    common = {
        "kpk": kpk,
        "vals": np.ascontiguousarray(values).astype(ml_dtypes.bfloat16),
        "wqT": wqp,
        "wg": wgp,
        "wo": wop,
        "shof": shof,
    }
    in_maps = []
    for c in range(N_CORES):
        xc = xf[c * cfg.T:(c + 1) * cfg.T]        # [T, dm]
        xTc = np.ascontiguousarray(xc.T)          # [dm, T]
        xp = np.ascontiguousarray(
            xTc.reshape(n_dm, 128, cfg.T).transpose(1, 0, 2))
        m = dict(common)
        m["xT"] = xp
        in_maps.append(m)
    return in_maps


def kernel(x, keys, values, w_q, w_gate, w_out):
    cfg = FULL
    if "nc" not in _CACHE:
        _CACHE["nc"] = build(cfg)
    nc = _CACHE["nc"]
    x = np.asarray(x)
    in_maps = _prep(x, np.asarray(keys), np.asarray(values),
                    np.asarray(w_q), np.asarray(w_gate), np.asarray(w_out),
                    cfg)
    trace = os.environ.get("KERNEL_TRACE", "0") == "1"
    if trace:
        try:
            import ntff_shim
            ntff_shim.install()
        except Exception:
            pass
    res = run_bass_kernel_spmd(nc, in_maps, list(range(N_CORES)), trace=trace)
    if trace:
        _CACHE["exec_time_ns"] = res.exec_time_ns
    outs = [res.results[c]["out"] for c in range(N_CORES)]
    B, S, D = x.shape
    return np.concatenate(outs, axis=0).reshape(B, S, D)
